# revision 7
# baseline (speedup 1.0000x reference)
"""ChannelMerger kernel for Trainium2, data-parallel over batch on 8 NeuronCores.

Reference computation (per batch b):
    pos       = layout + 0.2                              # [C, 2]
    loc[c,ij] = (2*pi/1.4) * (i * pos_x[c] + j * pos_y[c])   (i = ij>>5, j = ij&31)
    emb       = [cos(loc), sin(loc)]                      # [C, D=2048]
    scores    = emb @ heads.T                             # -> [O, C]
    weights   = softmax(scores, axis=C)
    out[b]    = weights @ x[b]                            # [O, T]

Device program (identical on all 8 cores, each owns 8 batches):
  phase 1 (replicated): embT [d, c] built directly in transposed layout via
    fractional-turn range reduction + ACT Sin; heads transposed on the PE;
    scoresT = embT.T @ headsT -> [c, o]; expT = exp(scoresT) (unnormalized
    softmax, f32r); per-o sums via ones-matmul; recip[o] = 1/sum.
  phase 2: out[b] = (expT.T @ x[b]) * recip[o]  -- fp32r matmuls, N-chunks of
    512 so each matmul stays inside one PSUM bank.
"""

import sys

for _p in ("/opt/trn_rl_repo", "/root/.axon_site/_ro/trn_rl_repo"):
    if _p not in sys.path:
        sys.path.append(_p)

import numpy as np

B, C, T = 64, 270, 2000
O, D = 270, 2048
N_CORES = 8
B_LOC = B // N_CORES          # 8 batches per core
NF = 32                       # fourier freqs per axis; NF*NF = 1024 = D//2
MARGIN = 0.2
WIDTH = 1.0 + 2.0 * MARGIN    # 1.4

# chunkings
C_CHUNKS = [(0, 128), (128, 128), (256, 14)]    # c (contraction) and o (output rows)
D_CHUNKS = 16                                   # 2048 / 128
IJ_CHUNKS = 8                                   # 1024 / 128
N_TILE = 512                                    # psum-bank-aligned t chunks
N_CHUNKS = [(0, 512), (512, 512), (1024, 512), (1536, 464)]

_cache = {}


def _build():
    import concourse.tile as tile
    from concourse import bacc, mybir
    from concourse.masks import make_identity

    F32 = mybir.dt.float32
    F32R = mybir.dt.float32r
    I32 = mybir.dt.int32
    ACT = mybir.ActivationFunctionType
    ALU = mybir.AluOpType
    TWO_PI = float(2.0 * np.pi)

    nc = bacc.Bacc("TRN2", target_bir_lowering=False, debug=False,
                   num_devices=N_CORES)

    x_ap = nc.dram_tensor("x", [B_LOC, C, T], F32, kind="ExternalInput").ap()
    lay_ap = nc.dram_tensor("layout", [C, 2], F32, kind="ExternalInput").ap()
    heads_ap = nc.dram_tensor("heads", [O, D], F32, kind="ExternalInput").ap()
    # ijc[:, k] = float((k*128 + p) >> 5) for k < 8; ijc[:, 8] = float(p & 31);
    # ijc[:, 9] = 1.0
    ijc_ap = nc.dram_tensor("ijc", [128, 10], F32, kind="ExternalInput").ap()
    out_ap = nc.dram_tensor("out", [B_LOC, O, T], F32, kind="ExternalOutput").ap()

    with tile.TileContext(nc) as tc:
        with tc.tile_pool(name="const", bufs=1) as cpool, \
             tc.tile_pool(name="expT", bufs=1) as epool:

            ident = cpool.tile([128, 128], F32)
            make_identity(nc, ident[:])
            ijc = cpool.tile([128, 10], F32)
            nc.sync.dma_start(ijc[:], ijc_ap[:])

            # pos rows -> scaled turn coefficients u = (pos_x+0.2)/1.4, v likewise
            posx = cpool.tile([1, C], F32)
            posy = cpool.tile([1, C], F32)
            nc.sync.dma_start(posx[:], lay_ap[:, 0])
            nc.sync.dma_start(posy[:], lay_ap[:, 1])
            u_row = cpool.tile([1, C], F32)
            nc.vector.tensor_scalar(u_row[:], posx[:], MARGIN, 1.0 / WIDTH,
                                    ALU.add, ALU.mult)
            v_row = cpool.tile([1, C], F32)
            nc.vector.tensor_scalar(v_row[:], posy[:], MARGIN, 1.0 / WIDTH,
                                    ALU.add, ALU.mult)
            u_bc = cpool.tile([128, C], F32)
            nc.gpsimd.partition_broadcast(u_bc[:], u_row[:])
            v_bc = cpool.tile([128, C], F32)
            nc.gpsimd.partition_broadcast(v_bc[:], v_row[:])

            # long-lived phase-1 outputs
            expT = [epool.tile([128, C], F32R, tag=f"expT{i}", name=f"expT{i}") for i in range(3)]
            recip = epool.tile([128, 4], F32)

            with tc.tile_pool(name="embT", bufs=1) as embpool, \
                 tc.tile_pool(name="headsT", bufs=1) as htpool:

                embT = [embpool.tile([128, C], F32R, tag=f"embT{i}", name=f"embT{i}")
                        for i in range(D_CHUNKS)]
                headsT = [htpool.tile([128, O], F32R, tag=f"headsT{i}", name=f"headsT{i}")
                          for i in range(D_CHUNKS)]

                # ---- embedding, transposed: embT[k][p, c] ----
                with tc.tile_pool(name="emb_work", bufs=2) as wpool:
                    for k in range(IJ_CHUNKS):
                        i_col = ijc[:, k:k + 1]
                        j_col = ijc[:, 8:9]
                        t2 = wpool.tile([128, C], F32, tag="t2")
                        nc.vector.tensor_scalar(t2[:], v_bc[:], j_col, None,
                                                ALU.mult)
                        f = wpool.tile([128, C], F32, tag="f")
                        nc.vector.scalar_tensor_tensor(
                            f[:], u_bc[:], i_col, t2[:], ALU.mult, ALU.add)
                        # sin chunk: emb[:, 1024 + k*128 : ...] = sin(2*pi*f)
                        ki = wpool.tile([128, C], I32, tag="ki")
                        nc.vector.tensor_copy(ki[:], f[:])
                        kf = wpool.tile([128, C], F32, tag="kf")
                        nc.vector.tensor_copy(kf[:], ki[:])
                        fs = wpool.tile([128, C], F32, tag="fs")
                        nc.vector.tensor_tensor(fs[:], f[:], kf[:], ALU.subtract)
                        nc.scalar.activation(embT[8 + k][:], fs[:], ACT.Sin,
                                             scale=TWO_PI)
                        # cos chunk: cos(2*pi*f) = sin(2*pi*(f+0.25))
                        g = wpool.tile([128, C], F32, tag="g")
                        nc.vector.tensor_scalar(g[:], f[:], 0.25, None, ALU.add)
                        gi = wpool.tile([128, C], I32, tag="gi")
                        nc.vector.tensor_copy(gi[:], g[:])
                        gf = wpool.tile([128, C], F32, tag="gf")
                        nc.vector.tensor_copy(gf[:], gi[:])
                        gs = wpool.tile([128, C], F32, tag="gs")
                        nc.vector.tensor_tensor(gs[:], g[:], gf[:], ALU.subtract)
                        nc.scalar.activation(embT[k][:], gs[:], ACT.Sin,
                                             scale=TWO_PI)

                # ---- transpose heads on the PE: headsT[dc][d, o] ----
                with tc.tile_pool(name="heads_in", bufs=1) as hpool, \
                     tc.tile_pool(name="tp_psum", bufs=4, space="PSUM") as tpp:
                    for oc, (o0, osz) in enumerate(C_CHUNKS):
                        hsb = hpool.tile([128, D], F32, tag=f"heads{oc}")
                        nc.sync.dma_start(hsb[:osz, :],
                                          heads_ap[o0:o0 + osz, :])
                        for dc in range(D_CHUNKS):
                            pt = tpp.tile([128, 128], F32, tag="tp")
                            nc.tensor.transpose(
                                pt[:, :osz],
                                hsb[:osz, dc * 128:(dc + 1) * 128],
                                ident[:osz, :osz])
                            nc.vector.tensor_copy(
                                headsT[dc][:, o0:o0 + osz], pt[:, :osz])

                # ---- scoresT = embT.T @ headsT ; expT = exp(scoresT) ----
                with tc.tile_pool(name="sc_psum", bufs=2, space="PSUM") as scp, \
                     tc.tile_pool(name="sum_psum", bufs=1, space="PSUM") as sup, \
                     tc.tile_pool(name="sum_work", bufs=1) as swp:
                    expF = [swp.tile([128, C], F32, tag=f"expF{i}",
                                     name=f"expF{i}") for i in range(3)]
                    for cc, (c0, csz) in enumerate(C_CHUNKS):
                        ps = scp.tile([128, O], F32, tag="sc")
                        for dc in range(D_CHUNKS):
                            nc.tensor.matmul(ps[:csz, :],
                                             embT[dc][:, c0:c0 + csz],
                                             headsT[dc][:],
                                             start=(dc == 0),
                                             stop=(dc == D_CHUNKS - 1))
                        nc.scalar.activation(expF[cc][:csz, :], ps[:csz, :],
                                             ACT.Exp)
                        nc.vector.tensor_copy(expT[cc][:csz, :],
                                              expF[cc][:csz, :])
                    # sums over c for each o-chunk (plain fp32), then recip
                    for oc, (o0, osz) in enumerate(C_CHUNKS):
                        ps = sup.tile([128, 1], F32, tag="sum")
                        for cc, (c0, csz) in enumerate(C_CHUNKS):
                            nc.tensor.matmul(ps[:osz, :],
                                             expF[cc][:csz, o0:o0 + osz],
                                             ijc[:csz, 9:10],
                                             start=(cc == 0), stop=(cc == 2))
                        nc.vector.reciprocal(recip[:osz, oc:oc + 1],
                                             ps[:osz, :])

            # ---- phase 2: out[b] = (expT.T @ x[b]) * recip ----
            with tc.tile_pool(name="xin", bufs=2) as xpool, \
                 tc.tile_pool(name="oout", bufs=3) as opool, \
                 tc.tile_pool(name="mm_psum", bufs=2, space="PSUM") as mmp:
                for b in range(B_LOC):
                    xb = []
                    for cc, (c0, csz) in enumerate(C_CHUNKS):
                        xt = xpool.tile([128, T], F32R, tag=f"x{cc}", name=f"x{cc}")
                        nc.sync.dma_start(
                            xt[:csz, :],
                            x_ap[b, c0:c0 + csz, :].bitcast(F32R))
                        xb.append(xt)
                    for oc, (o0, osz) in enumerate(C_CHUNKS):
                        pm = mmp.tile([128, 2048], F32, tag="mm")
                        for (n0, nsz) in N_CHUNKS:
                            for cc, (c0, csz) in enumerate(C_CHUNKS):
                                nc.tensor.matmul(
                                    pm[:osz, n0:n0 + nsz],
                                    expT[cc][:csz, o0:o0 + osz],
                                    xb[cc][:csz, n0:n0 + nsz],
                                    start=(cc == 0), stop=(cc == 2))
                        ot = opool.tile([128, T], F32, tag="o")
                        nc.vector.tensor_scalar(ot[:osz, :], pm[:osz, :T],
                                                recip[:osz, oc:oc + 1], None,
                                                ALU.mult)
                        nc.sync.dma_start(out_ap[b, o0:o0 + osz, :],
                                          ot[:osz, :])

    nc.compile()
    return nc


def _ijc_const():
    p = np.arange(128)
    cols = [((k * 128 + p) >> 5).astype(np.float32) for k in range(IJ_CHUNKS)]
    cols.append((p & 31).astype(np.float32))
    cols.append(np.ones(128, np.float32))
    return np.stack(cols, axis=1)


def get_nc():
    if "nc" not in _cache:
        _cache["nc"] = _build()
    return _cache["nc"]


def kernel(x, layout, heads):
    from concourse.bass_utils import run_bass_kernel_spmd

    assert x.shape == (B, C, T) and layout.shape == (C, 2)
    assert heads.shape == (O, D)
    nc = get_nc()
    ijc = _ijc_const()
    in_maps = [
        {
            "x": np.ascontiguousarray(x[m * B_LOC:(m + 1) * B_LOC]),
            "layout": np.ascontiguousarray(layout.astype(np.float32)),
            "heads": np.ascontiguousarray(heads.astype(np.float32)),
            "ijc": ijc,
        }
        for m in range(N_CORES)
    ]
    res = run_bass_kernel_spmd(nc, in_maps, list(range(N_CORES)))
    out = np.concatenate([res.results[m]["out"] for m in range(N_CORES)], axis=0)
    return out.astype(np.float32)
